# revision 38
# baseline (speedup 1.0000x reference)
"""Trainium2 Bass kernel: 4096x4096 single-channel 3x3 VALID conv + bias.

Sharding: 2x4 spatial grid over 8 cores. Core c = (rb, cb) = (c//4, c%4)
computes output block rows [2047*rb, +2047) x cols [1024*cb, +1024)
(cb=3: 1022 valid cols). Input shard: [2049, 1026] bf16 (halo included,
zero-padded right edge for cb=3). All I/O is bf16 (5e-3 rel err vs the
2e-2 budget); host converts fp32<->bf16.

Per core: 18 stripes of output rows [32, 126x15, 117, 8]. The first
stripe is small so its load lands early (DMA work is assigned to engines
in ~64KB chunks; one chunk is ~2.5us serial on one engine, so the first
compute-gating load must be tiny and split across queues). The last
stripe is 8 rows so the final store (16KB) drains in ~0.6us instead of
2.5. Early loads are split across the sync/scalar/vector queues to land
in parallel; gpsimd issues the band matrices and all remaining loads
up-front (x/y tiles are all SBUF-resident; no reuse, no WAR hazards).
Stores go on the sync queue. PSUM bank-pairs rotate mod 4; ScalarE (576
cols) + VectorE (448 cols) evacuate each stripe fusing +bias and bf16
cast. 2 warmup matmuls on the early band piece start the PE p-state ramp
before stripe 0's data lands; a dummy 1-elem activation pre-triggers the
ACT table load.

Sync notes (hard-won):
- At most ONE sem wait and ONE sem update per instruction; extra waits
  are standalone engine.wait_ge() instructions.
- An HWDGE dma_start does NOT wait for prior compute writes; gate DMAs
  reading compute results on a sem incremented by the producer.
- Load-completion sems use 4 rotating in-sems; pieces sharing a sem are
  >=4 stripes apart (~4us of ring work), so value-gating cannot invert.
"""

import numpy as np

import concourse.bass as bass
import concourse.mybir as mybir
from concourse.bass_utils import run_bass_kernel_spmd

H = W = 4096
KH = KW = 3
OH = OW = H - KH + 1   # 4094
NCORES = 8
GRID_R, GRID_C = 2, 4
BROWS = 2047           # output rows per core block
BCOLS = 1024           # output cols per core block (cb=3: 1022 valid)
IN_ROWS = BROWS + KH - 1   # 2049
IN_COLS = BCOLS + KW - 1   # 1026
SPLIT = 512            # scalar evacuates [0:512), vector [512:1024) (bank-aligned)

# stripe heights: 16 full stripes + 31-row last (its store is split by
# evac half across the two store rings to shorten the drain)
HEIGHTS = [126] * 16 + [31]
assert sum(HEIGHTS) == BROWS
STRIPES = []           # (out_row_start, out_rows, in_rows)
_r = 0
for _h in HEIGHTS:
    STRIPES.append((_r, _h, _h + KH - 1))
    _r += _h
N_S = len(STRIPES)     # 18

# Load plumbing, learned from sem-counter traces:
# - a DMA's completion-sem lags ~1-2.5us extra when many dma_starts are
#   queued behind it on the same ring, so the stripe-0 gating pieces go
#   on rings that stay otherwise empty (sync/scalar; their stores/evacs
#   are sem-gated far later);
# - each ring round-robins its ~64KB chunks over DMA engines starting at
#   engine 0, so rings collide there; tiny dummy loads offset the
#   pointers (sync +1, scalar +2) to keep the first pieces parallel;
# - gpsimd's ring carries mb_a first (completes before congestion), then
#   the bulk in need-order.
# stripe 0+1 input is packed into the mb tensor (one combined load on the
# otherwise-quiet sync ring gates stripes 0-1 with a single completion).
MBX_X0 = KW * 126            # 378: x0 rows live at mb[0:34, 378:1404]
MBX_X1 = MBX_X0 + IN_COLS    # 1404: x1 rows at mb[0:128, 1404:2430]
MBX_COLS = MBX_X1 + IN_COLS  # 2430
Q_SYNC2 = [(2, 0, 64), (2, 64, 128)]

_cached = None


def _in_sem_thresholds():
    """Cumulative then_inc(16) totals per rotating in-sem, per stripe."""
    cum = [0, 0, 0, 0]
    thresh = [0] * N_S
    pieces = {s: 0 for s in range(N_S)}
    for s, _, _ in Q_SYNC2:
        pieces[s] += 1
    for s in range(3, N_S):
        pieces[s] += 1
    for s in range(N_S):
        cum[s % 4] += 16 * pieces[s]
        thresh[s] = cum[s % 4]
    return thresh


def _build():
    nc = bass.Bass()
    x_d = nc.dram_tensor("x", [IN_ROWS, IN_COLS], mybir.dt.bfloat16, kind="ExternalInput")
    mb_d = nc.dram_tensor("mb", [128, MBX_COLS], mybir.dt.bfloat16, kind="ExternalInput")
    bv_d = nc.dram_tensor("bv", [128, 1], mybir.dt.float32, kind="ExternalInput")
    y_d = nc.dram_tensor("y", [BROWS, BCOLS], mybir.dt.bfloat16, kind="ExternalOutput")

    thresh = _in_sem_thresholds()

    import contextlib
    with contextlib.ExitStack() as st:
        ec = st.enter_context
        xb = [None, None] + [
            ec(nc.sbuf_tensor(f"x{s}", [128, IN_COLS], mybir.dt.bfloat16))
            for s in range(2, N_S)]
        yb = [ec(nc.sbuf_tensor(f"y{s}", [128, BCOLS], mybir.dt.bfloat16))
              for s in range(N_S)]
        mb = ec(nc.sbuf_tensor("mb_sb", [128, MBX_COLS], mybir.dt.bfloat16))
        bv = ec(nc.sbuf_tensor("bv_sb", [128, 1], mybir.dt.float32))
        scr = ec(nc.sbuf_tensor("scr", [1, 8], mybir.dt.float32))
        ps = ec(nc.psum_tensor([128, 4096], mybir.dt.float32))
        wm = ec(nc.semaphore("wm"))
        ldA = ec(nc.semaphore("ldA"))
        ldbv = ec(nc.semaphore("ldbv"))
        ins = [ec(nc.semaphore(f"in{q}")) for q in range(4)]
        pe_sem = ec(nc.semaphore("pe_sem"))
        evA = ec(nc.semaphore("evA"))
        evB = ec(nc.semaphore("evB"))
        st_sem = ec(nc.semaphore("st_sem"))
        blk = ec(nc.Block())

        def load_piece(eng, s, lo, hi):
            r0 = STRIPES[s][0]
            eng.dma_start(
                xb[s][lo:hi, :], x_d.ap()[r0 + lo:r0 + hi, :]
            ).then_inc(ins[s % 4], 16)

        @blk.gpsimd
        def _(gpsimd):
            gpsimd.dma_start(bv[:], bv_d.ap()).then_inc(ldbv, 16)
            # let the combined mb+x0+x1 load claim engines first: its ring
            # chunks race this ring's bulk for the shared DMA engines
            gpsimd.wait_ge(ldA, 8)
            for s in range(3, N_S):
                load_piece(gpsimd, s, 0, STRIPES[s][2])
            for s, (r0, orows, irows) in enumerate(STRIPES):
                if s % 2 != 0:
                    continue
                gpsimd.wait_ge(evA, s + 1)
                if s == N_S - 1:
                    # row-split across rings: full-width 2KB lines move at
                    # ~80ns/desc vs ~155ns for half-width 1KB lines
                    gpsimd.wait_ge(evB, s + 1)
                    gpsimd.dma_start(
                        y_d.ap()[r0 + 16:r0 + orows, :], yb[s][16:orows, 0:BCOLS]
                    ).then_inc(st_sem, 16)
                else:
                    gpsimd.wait_ge(evB, s + 1)
                    gpsimd.dma_start(
                        y_d.ap()[r0:r0 + orows, :], yb[s][0:orows, 0:BCOLS]
                    ).then_inc(st_sem, 16)

        @blk.scalar
        def _(scalar):
            scalar.wait_ge(ldbv, 16)
            # trigger the lazy ACT table load before the first real evac
            nc.scalar.activation(
                out=scr[0:1, 3:4], in_=bv[0:1, 0:1],
                func=mybir.ActivationFunctionType.Identity,
                bias=bv[0:1, 0:1], scale=1.0,
            )
            for s, (r0, orows, irows) in enumerate(STRIPES):
                p = s % 4
                # bank A (cols 0:512 of the pair) is complete at 2s+1;
                # waking here overlaps the park-wakeup with bank B's MMs
                scalar.wait_ge(pe_sem, 2 * s + 1)
                nc.scalar.activation(
                    out=yb[s][0:orows, 0:SPLIT],
                    in_=ps[0:orows, 1024 * p:1024 * p + SPLIT],
                    func=mybir.ActivationFunctionType.Identity,
                    bias=bv[0:orows, 0:1],
                    scale=1.0,
                ).then_inc(evA, 1)
            # last stripe's bank B too: scalar is already running (no
            # park-wakeup), vector would wake ~1.3us after the final MM
            s, (r0, orows, irows) = N_S - 1, STRIPES[N_S - 1]
            p = s % 4
            scalar.wait_ge(pe_sem, 2 * s + 2)
            nc.scalar.activation(
                out=yb[s][0:orows, SPLIT:BCOLS],
                in_=ps[0:orows, 1024 * p + SPLIT:1024 * p + BCOLS],
                func=mybir.ActivationFunctionType.Identity,
                bias=bv[0:orows, 0:1],
                scale=1.0,
            ).then_inc(evB, 1)

        @blk.vector
        def _(vector):
            # zero scratch for the PE's data-independent p-state warmup
            nc.vector.memset(yb[N_S - 1][0:128, 0:512], 0).then_inc(wm, 1)
            vector.wait_ge(ldbv, 16)
            for s, (r0, orows, irows) in enumerate(STRIPES):
                if s == N_S - 1:
                    continue
                p = s % 4
                vector.wait_ge(pe_sem, 2 * s + 2)
                nc.vector.tensor_scalar_add(
                    out=yb[s][0:orows, SPLIT:BCOLS],
                    in0=ps[0:orows, 1024 * p + SPLIT:1024 * p + BCOLS],
                    scalar1=bv[0:orows, 0:1],
                ).then_inc(evB, 1)

        @blk.tensor
        def _(tensor):
            tensor.wait_ge(wm, 1)
            # p-state warmup on zeroed scratch while the combined load
            # lands; the last rounds are paced by its progress increments
            # so the PE never idles (an idle gap resets the clock ramp)
            def warm(n):
                for _ in range(n):
                    nc.tensor.matmul(
                        ps[0:126, 2048:2560],
                        yb[N_S - 1][0:128, 0:126],
                        yb[N_S - 1][0:128, 0:512],
                        start=True, stop=True,
                    )
            warm(10)
            tensor.wait_ge(ldA, 16)
            for s, (r0, orows, irows) in enumerate(STRIPES):
                p = s % 4
                if thresh[s]:
                    tensor.wait_ge(ins[p], thresh[s])
                if s >= 4:
                    tensor.wait_ge(evA, s - 3)
                    tensor.wait_ge(evB, s - 3)
                if s == 0:
                    rhs, rc = mb, MBX_X0
                elif s == 1:
                    rhs, rc = mb, MBX_X1
                else:
                    rhs, rc = xb[s], 0
                for h in range(2):
                    c0 = 1024 * p + 512 * h
                    mm = None
                    for dj in range(KW):
                        mm = nc.tensor.matmul(
                            ps[0:orows, c0:c0 + 512],
                            mb[0:irows, dj * 126:dj * 126 + orows],
                            rhs[0:irows, rc + 512 * h + dj:rc + 512 * h + dj + 512],
                            start=(dj == 0),
                            stop=(dj == KW - 1),
                        )
                    mm.then_inc(pe_sem, 1)

        @blk.sync
        def _(sync):
            # dummy load shifts this ring's engine pointer off engine 0
            sync.dma_start(mb[:], mb_d.ap()).then_inc(ldA, 16)
            for s, lo, hi in Q_SYNC2:
                load_piece(sync, s, lo, hi)
            for s, (r0, orows, irows) in enumerate(STRIPES):
                if s % 2 == 0 and s != N_S - 1:
                    continue
                if s == N_S - 1:
                    sync.wait_ge(evA, s + 1)
                    sync.wait_ge(evB, s + 1)
                    sync.dma_start(
                        y_d.ap()[r0:r0 + 16, :], yb[s][0:16, 0:BCOLS]
                    ).then_inc(st_sem, 16)
                else:
                    sync.wait_ge(evA, s + 1)
                    sync.wait_ge(evB, s + 1)
                    sync.dma_start(
                        y_d.ap()[r0:r0 + orows, :], yb[s][0:orows, 0:BCOLS]
                    ).then_inc(st_sem, 16)
            # hold the NEFF open until all stores land
            sync.wait_ge(st_sem, 16 * (N_S + 1))

    return nc


def _host_prep(input, weight, bias):
    import ml_dtypes
    bf16 = ml_dtypes.bfloat16
    input = np.ascontiguousarray(np.asarray(input, dtype=np.float32).astype(bf16))
    weight = np.asarray(weight, dtype=np.float32)
    bias = np.asarray(bias, dtype=np.float32)

    # band matrices packed side by side: mb[:, dj*126+m] column m of M_dj,
    # M_dj[k, m] = weight[k-m, dj] for 0 <= k-m < KH; stripe 0+1 input is
    # appended per core so one load gates the first two stripes.
    band = np.zeros((128, KW * 126), dtype=np.float32)
    idx = np.arange(126)
    for dj in range(KW):
        for di in range(KH):
            band[idx + di, dj * 126 + idx] = weight[di, dj]
    band = band.astype(bf16)
    bv = np.full((128, 1), bias[0], dtype=np.float32)

    in_maps = []
    for c in range(NCORES):
        rb, cb = c // GRID_C, c % GRID_C
        r0, c0 = rb * BROWS, cb * BCOLS
        sl = input[r0:r0 + IN_ROWS, c0:c0 + IN_COLS]
        if sl.shape[1] < IN_COLS:
            sl = np.concatenate(
                [sl, np.zeros((sl.shape[0], IN_COLS - sl.shape[1]), bf16)], axis=1
            )
        sl = np.ascontiguousarray(sl)
        mbx = np.zeros((128, MBX_COLS), dtype=bf16)
        mbx[:, :KW * 126] = band
        mbx[0:STRIPES[0][2], MBX_X0:MBX_X0 + IN_COLS] = sl[0:STRIPES[0][2]]
        r1 = STRIPES[1][0]
        mbx[0:STRIPES[1][2], MBX_X1:MBX_X1 + IN_COLS] = sl[r1:r1 + STRIPES[1][2]]
        in_maps.append({"x": sl, "mb": mbx, "bv": bv})
    return in_maps


def _run(input, weight, bias, **spmd_kwargs):
    global _cached
    if _cached is None:
        _cached = _build()
    in_maps = _host_prep(input, weight, bias)
    res = run_bass_kernel_spmd(
        _cached, in_maps, core_ids=list(range(NCORES)), **spmd_kwargs
    )
    out = np.empty((OH, OW), dtype=np.float32)
    for c in range(NCORES):
        rb, cb = c // GRID_C, c % GRID_C
        r0, c0 = rb * BROWS, cb * BCOLS
        cols = min(BCOLS, OW - c0)
        out[r0:r0 + BROWS, c0:c0 + cols] = (
            res.results[c]["y"][:, :cols].astype(np.float32)
        )
    return out, res


def kernel(input, weight, bias):
    out, _ = _run(input, weight, bias)
    return out


# revision 39
# speedup vs baseline: 1.0233x; 1.0233x over previous
"""Trainium2 Bass kernel: 4096x4096 single-channel 3x3 VALID conv + bias.

Sharding: 2x4 spatial grid over 8 cores. Core c = (rb, cb) = (c//4, c%4)
computes output rows [2047*rb, +2047) x cols [1024*cb, +1024) (cb=3: 1022
valid). Input shard: [2049, 1026] bf16 (halo included, zero-padded right
edge for cb=3). All I/O is bf16 (5e-3 rel err vs the 2e-2 budget); host
converts fp32<->bf16. The 2x4 grid keeps DMA lines at 2KB+ (efficient
descriptors) and PE column sweeps near-minimal (17 stripes x 3 shifts x
1024 cols ~ 52k cycles ~ 21.8us at 2.4GHz).

Per core: 17 stripes of <=126 output rows, all x/y tiles SBUF-resident
(no reuse, no WAR hazards). Per stripe, per 512-col PSUM bank (pairs
rotate mod 4), 3 bf16 matmuls (kernel column dj, rhs shifted by dj)
against 128x126 band matrices accumulate all 9 taps. ScalarE/VectorE
evacuate the two banks fusing +bias and the bf16 cast; stores alternate
between the sync and gpsimd rings.

Start path: stripe 0+1's input is packed into the band-matrix tensor
(columns 378:2430), so ONE combined load on the otherwise-idle sync ring
gates the first two stripes with a single completion; the PE runs
data-independent warmup matmuls on a zeroed scratch region until it
lands (sized so there is NO idle gap - an idle PE resets the DVFS ramp
and costs ~2-4us of half-speed matmuls).

Hard-won DMA facts (from semaphore-counter traces):
- dma_start costs ~0.65-0.8us of engine issue time on any queue.
- A ring hands work to the 16 DMA engines in ~64KB chunks, round-robin
  from engine 0, so any single load has a ~2.5us transfer wall and
  parallel rings collide head-on; the bulk ring (gpsimd) is gated on the
  combined load's progress (ldA>=8) so it cannot steal its engines.
- then_inc(sem,16) posts ~15 progress increments at descriptor-gen time
  and the 16th at true completion; completion lags ~0.7-2.5us more when
  many dma_starts are queued behind on the same ring. Gating waits must
  be exact multiples of 16 per contributing DMA.
- Full-width 2KB store lines move at ~80ns/descriptor; half-width 1KB
  (column-split) lines cost ~155ns - never split stores by columns.
  The last stripe's store is split by ROWS across both rings instead.
- Compute engines cannot read across a 512-float PSUM bank boundary
  (device hangs); evac ops are bank-aligned.
- A parked engine takes ~1.3us to wake from a semaphore wait; ScalarE
  pre-wakes on the even bank (pe>=2s+1) and also handles the last
  stripe's odd bank (it is already running; VectorE would park).

Sync rules: at most ONE sem wait and ONE sem update per instruction
(extra waits are standalone); HWDGE dma_starts reading compute results
must be sem-gated on the producer; rotating in0..in3 sems are safe
because pieces sharing a sem are >=4 stripes (~4us of ring work) apart.
"""

import numpy as np

import concourse.bass as bass
import concourse.mybir as mybir
from concourse.bass_utils import run_bass_kernel_spmd

H = W = 4096
KH = KW = 3
OH = OW = H - KH + 1   # 4094
NCORES = 8
GRID_R, GRID_C = 2, 4
BROWS = 2047           # output rows per core block
BCOLS = 1024           # output cols per core block (cb=3: 1022 valid)
IN_ROWS = BROWS + KH - 1   # 2049
IN_COLS = BCOLS + KW - 1   # 1026
SPLIT = 512            # scalar evacuates [0:512), vector [512:1024) (bank-aligned)

# stripe heights: 16 full stripes + 31-row last (its store is split by
# evac half across the two store rings to shorten the drain)
HEIGHTS = [126] * 16 + [31]
assert sum(HEIGHTS) == BROWS
STRIPES = []           # (out_row_start, out_rows, in_rows)
_r = 0
for _h in HEIGHTS:
    STRIPES.append((_r, _h, _h + KH - 1))
    _r += _h
N_S = len(STRIPES)     # 18

# Load plumbing, learned from sem-counter traces:
# - a DMA's completion-sem lags ~1-2.5us extra when many dma_starts are
#   queued behind it on the same ring, so the stripe-0 gating pieces go
#   on rings that stay otherwise empty (sync/scalar; their stores/evacs
#   are sem-gated far later);
# - each ring round-robins its ~64KB chunks over DMA engines starting at
#   engine 0, so rings collide there; tiny dummy loads offset the
#   pointers (sync +1, scalar +2) to keep the first pieces parallel;
# - gpsimd's ring carries mb_a first (completes before congestion), then
#   the bulk in need-order.
# stripe 0+1 input is packed into the mb tensor (one combined load on the
# otherwise-quiet sync ring gates stripes 0-1 with a single completion).
MBX_X0 = KW * 126            # 378: x0 rows live at mb[0:34, 378:1404]
MBX_X1 = MBX_X0 + IN_COLS    # 1404: x1 rows at mb[0:128, 1404:2430]
MBX_COLS = MBX_X1 + IN_COLS  # 2430
Q_SYNC2 = [(2, 0, 64), (2, 64, 128)]

_cached = None


def _in_sem_thresholds():
    """Cumulative then_inc(16) totals per rotating in-sem, per stripe."""
    cum = [0, 0, 0, 0]
    thresh = [0] * N_S
    pieces = {s: 0 for s in range(N_S)}
    for s, _, _ in Q_SYNC2:
        pieces[s] += 1
    for s in range(3, N_S):
        pieces[s] += 1
    for s in range(N_S):
        cum[s % 4] += 16 * pieces[s]
        thresh[s] = cum[s % 4]
    return thresh


def _build():
    nc = bass.Bass()
    x_d = nc.dram_tensor("x", [IN_ROWS, IN_COLS], mybir.dt.bfloat16, kind="ExternalInput")
    mb_d = nc.dram_tensor("mb", [128, MBX_COLS], mybir.dt.bfloat16, kind="ExternalInput")
    bv_d = nc.dram_tensor("bv", [128, 1], mybir.dt.float32, kind="ExternalInput")
    y_d = nc.dram_tensor("y", [BROWS, BCOLS], mybir.dt.bfloat16, kind="ExternalOutput")

    thresh = _in_sem_thresholds()

    import contextlib
    with contextlib.ExitStack() as st:
        ec = st.enter_context
        xb = [None, None] + [
            ec(nc.sbuf_tensor(f"x{s}", [128, IN_COLS], mybir.dt.bfloat16))
            for s in range(2, N_S)]
        yb = [ec(nc.sbuf_tensor(f"y{s}", [128, BCOLS], mybir.dt.bfloat16))
              for s in range(N_S)]
        mb = ec(nc.sbuf_tensor("mb_sb", [128, MBX_COLS], mybir.dt.bfloat16))
        bv = ec(nc.sbuf_tensor("bv_sb", [128, 1], mybir.dt.float32))
        scr = ec(nc.sbuf_tensor("scr", [1, 8], mybir.dt.float32))
        ps = ec(nc.psum_tensor([128, 4096], mybir.dt.float32))
        wm = ec(nc.semaphore("wm"))
        ldA = ec(nc.semaphore("ldA"))
        ldbv = ec(nc.semaphore("ldbv"))
        ins = [ec(nc.semaphore(f"in{q}")) for q in range(4)]
        pe_sem = ec(nc.semaphore("pe_sem"))
        evA = ec(nc.semaphore("evA"))
        evB = ec(nc.semaphore("evB"))
        st_sem = ec(nc.semaphore("st_sem"))
        blk = ec(nc.Block())

        def load_piece(eng, s, lo, hi):
            r0 = STRIPES[s][0]
            eng.dma_start(
                xb[s][lo:hi, :], x_d.ap()[r0 + lo:r0 + hi, :]
            ).then_inc(ins[s % 4], 16)

        @blk.gpsimd
        def _(gpsimd):
            gpsimd.dma_start(bv[:], bv_d.ap()).then_inc(ldbv, 16)
            # let the combined mb+x0+x1 load claim engines first: its ring
            # chunks race this ring's bulk for the shared DMA engines
            gpsimd.wait_ge(ldA, 8)
            for s in range(3, N_S):
                load_piece(gpsimd, s, 0, STRIPES[s][2])
            for s, (r0, orows, irows) in enumerate(STRIPES):
                if s % 2 != 0:
                    continue
                gpsimd.wait_ge(evA, s + 1)
                if s == N_S - 1:
                    # row-split across rings: full-width 2KB lines move at
                    # ~80ns/desc vs ~155ns for half-width 1KB lines
                    gpsimd.wait_ge(evB, s + 1)
                    gpsimd.dma_start(
                        y_d.ap()[r0 + 16:r0 + orows, :], yb[s][16:orows, 0:BCOLS]
                    ).then_inc(st_sem, 16)
                else:
                    gpsimd.wait_ge(evB, s + 1)
                    gpsimd.dma_start(
                        y_d.ap()[r0:r0 + orows, :], yb[s][0:orows, 0:BCOLS]
                    ).then_inc(st_sem, 16)

        @blk.scalar
        def _(scalar):
            scalar.wait_ge(ldbv, 16)
            # trigger the lazy ACT table load before the first real evac
            nc.scalar.activation(
                out=scr[0:1, 3:4], in_=bv[0:1, 0:1],
                func=mybir.ActivationFunctionType.Identity,
                bias=bv[0:1, 0:1], scale=1.0,
            )
            for s, (r0, orows, irows) in enumerate(STRIPES):
                p = s % 4
                # bank A (cols 0:512 of the pair) is complete at 2s+1;
                # waking here overlaps the park-wakeup with bank B's MMs
                scalar.wait_ge(pe_sem, 2 * s + 1)
                nc.scalar.activation(
                    out=yb[s][0:orows, 0:SPLIT],
                    in_=ps[0:orows, 1024 * p:1024 * p + SPLIT],
                    func=mybir.ActivationFunctionType.Identity,
                    bias=bv[0:orows, 0:1],
                    scale=1.0,
                ).then_inc(evA, 1)
            # last stripe's bank B too: scalar is already running (no
            # park-wakeup), vector would wake ~1.3us after the final MM
            s, (r0, orows, irows) = N_S - 1, STRIPES[N_S - 1]
            p = s % 4
            scalar.wait_ge(pe_sem, 2 * s + 2)
            nc.scalar.activation(
                out=yb[s][0:orows, SPLIT:BCOLS],
                in_=ps[0:orows, 1024 * p + SPLIT:1024 * p + BCOLS],
                func=mybir.ActivationFunctionType.Identity,
                bias=bv[0:orows, 0:1],
                scale=1.0,
            ).then_inc(evB, 1)

        @blk.vector
        def _(vector):
            # zero scratch for the PE's data-independent p-state warmup
            nc.vector.memset(yb[N_S - 1][0:128, 0:512], 0).then_inc(wm, 1)
            vector.wait_ge(ldbv, 16)
            for s, (r0, orows, irows) in enumerate(STRIPES):
                if s == N_S - 1:
                    continue
                p = s % 4
                vector.wait_ge(pe_sem, 2 * s + 2)
                nc.vector.tensor_scalar_add(
                    out=yb[s][0:orows, SPLIT:BCOLS],
                    in0=ps[0:orows, 1024 * p + SPLIT:1024 * p + BCOLS],
                    scalar1=bv[0:orows, 0:1],
                ).then_inc(evB, 1)

        @blk.tensor
        def _(tensor):
            tensor.wait_ge(wm, 1)
            # p-state warmup on zeroed scratch while the combined load
            # lands; the last rounds are paced by its progress increments
            # so the PE never idles (an idle gap resets the clock ramp)
            def warm(n):
                for _ in range(n):
                    nc.tensor.matmul(
                        ps[0:126, 2048:2560],
                        yb[N_S - 1][0:128, 0:126],
                        yb[N_S - 1][0:128, 0:512],
                        start=True, stop=True,
                    )
            warm(10)
            tensor.wait_ge(ldA, 16)
            for s, (r0, orows, irows) in enumerate(STRIPES):
                p = s % 4
                if thresh[s]:
                    tensor.wait_ge(ins[p], thresh[s])
                if s >= 4:
                    tensor.wait_ge(evA, s - 3)
                    tensor.wait_ge(evB, s - 3)
                if s == 0:
                    rhs, rc = mb, MBX_X0
                elif s == 1:
                    rhs, rc = mb, MBX_X1
                else:
                    rhs, rc = xb[s], 0
                for h in range(2):
                    c0 = 1024 * p + 512 * h
                    mm = None
                    for dj in range(KW):
                        mm = nc.tensor.matmul(
                            ps[0:orows, c0:c0 + 512],
                            mb[0:irows, dj * 126:dj * 126 + orows],
                            rhs[0:irows, rc + 512 * h + dj:rc + 512 * h + dj + 512],
                            start=(dj == 0),
                            stop=(dj == KW - 1),
                        )
                    mm.then_inc(pe_sem, 1)

        @blk.sync
        def _(sync):
            # dummy load shifts this ring's engine pointer off engine 0
            sync.dma_start(mb[:], mb_d.ap()).then_inc(ldA, 16)
            for s, lo, hi in Q_SYNC2:
                load_piece(sync, s, lo, hi)
            for s, (r0, orows, irows) in enumerate(STRIPES):
                if s % 2 == 0 and s != N_S - 1:
                    continue
                if s == N_S - 1:
                    sync.wait_ge(evA, s + 1)
                    sync.wait_ge(evB, s + 1)
                    sync.dma_start(
                        y_d.ap()[r0:r0 + 16, :], yb[s][0:16, 0:BCOLS]
                    ).then_inc(st_sem, 16)
                else:
                    sync.wait_ge(evA, s + 1)
                    sync.wait_ge(evB, s + 1)
                    sync.dma_start(
                        y_d.ap()[r0:r0 + orows, :], yb[s][0:orows, 0:BCOLS]
                    ).then_inc(st_sem, 16)
            # hold the NEFF open until all stores land
            sync.wait_ge(st_sem, 16 * (N_S + 1))

    return nc


def _host_prep(input, weight, bias):
    import ml_dtypes
    bf16 = ml_dtypes.bfloat16
    input = np.ascontiguousarray(np.asarray(input, dtype=np.float32).astype(bf16))
    weight = np.asarray(weight, dtype=np.float32)
    bias = np.asarray(bias, dtype=np.float32)

    # band matrices packed side by side: mb[:, dj*126+m] column m of M_dj,
    # M_dj[k, m] = weight[k-m, dj] for 0 <= k-m < KH; stripe 0+1 input is
    # appended per core so one load gates the first two stripes.
    band = np.zeros((128, KW * 126), dtype=np.float32)
    idx = np.arange(126)
    for dj in range(KW):
        for di in range(KH):
            band[idx + di, dj * 126 + idx] = weight[di, dj]
    band = band.astype(bf16)
    bv = np.full((128, 1), bias[0], dtype=np.float32)

    in_maps = []
    for c in range(NCORES):
        rb, cb = c // GRID_C, c % GRID_C
        r0, c0 = rb * BROWS, cb * BCOLS
        sl = input[r0:r0 + IN_ROWS, c0:c0 + IN_COLS]
        if sl.shape[1] < IN_COLS:
            sl = np.concatenate(
                [sl, np.zeros((sl.shape[0], IN_COLS - sl.shape[1]), bf16)], axis=1
            )
        sl = np.ascontiguousarray(sl)
        mbx = np.zeros((128, MBX_COLS), dtype=bf16)
        mbx[:, :KW * 126] = band
        mbx[0:STRIPES[0][2], MBX_X0:MBX_X0 + IN_COLS] = sl[0:STRIPES[0][2]]
        r1 = STRIPES[1][0]
        mbx[0:STRIPES[1][2], MBX_X1:MBX_X1 + IN_COLS] = sl[r1:r1 + STRIPES[1][2]]
        in_maps.append({"x": sl, "mb": mbx, "bv": bv})
    return in_maps


def _run(input, weight, bias, **spmd_kwargs):
    global _cached
    if _cached is None:
        _cached = _build()
    in_maps = _host_prep(input, weight, bias)
    res = run_bass_kernel_spmd(
        _cached, in_maps, core_ids=list(range(NCORES)), **spmd_kwargs
    )
    out = np.empty((OH, OW), dtype=np.float32)
    for c in range(NCORES):
        rb, cb = c // GRID_C, c % GRID_C
        r0, c0 = rb * BROWS, cb * BCOLS
        cols = min(BCOLS, OW - c0)
        out[r0:r0 + BROWS, c0:c0 + cols] = (
            res.results[c]["y"][:, :cols].astype(np.float32)
        )
    return out, res


def kernel(input, weight, bias):
    out, _ = _run(input, weight, bias)
    return out
